# revision 15
# baseline (speedup 1.0000x reference)
"""ContactMapLinear Trainium2 kernel, v6.

res = tril((X @ P) @ (Q @ X^T), k=-1), X = features[0, 1:4097, :], 8-core SPMD.

v6 insight: a pure-MM microbenchmark measured ~112 ns per N=512 bf16
matmul (PE streams ~2 bf16 columns/cycle), so the kernel's PE floor is
~75 us/rep, not ~143 us -- v4's 137.6 us was DMA/collective-bound, not
compute-bound. v6 cuts steady-state HBM traffic and collective exposure:
  - P and Q weight tiles (16 MB/core) are constant across reps: loaded
    once on rep 0 into resident SBUF tiles (~128 KB/partition), no
    per-rep weight streaming.
  - Output written bf16 (host upcasts): 5.25 -> 2.6 MB/rep.
  - Phase B AllGathers B in 4 quarters (2 MB each) right after each pair
    of inner tiles, so every gather hides under the remaining ~50-29 us
    of compute (a 4 MB half-gather no longer fits under the shrunken
    phase A).
  - mask loaded rep 0 only.

  Phase B q=0..3: B[:, 2q:2q+2, :] = Qq @ Xcols_c^T,  AllGather quarter
  Phase A:        AT_c = P^T @ Xrows_c^T
  Phase C:        staircase S rows = AT^T @ B, strict-lower mask on diag.
"""

import sys

import ml_dtypes
import numpy as np

_TRN_REPO = "/opt/trn_rl_repo"
if _TRN_REPO not in sys.path:
    sys.path.insert(0, _TRN_REPO)

D = 4096          # seq length / feature dim
I = 1024          # inner dim
N_CORES = 8
R = D // N_CORES  # 512 seq rows per core
P = 128           # partitions
KT = D // P       # 32 feature k-tiles
IT = I // P       # 8 inner tiles
MT = R // P       # 4 row m-tiles per core
BF16 = ml_dtypes.bfloat16

_CACHE = {}


def _build(repeat: int = 1, sim: bool = False, bj_bufs: int = 2,
           oc_bufs: int = 3, xr_depth: int = 3, warmup_mms: int = 12,
           no_coll: bool = False):
    import concourse.mybir as mybir
    import concourse.tile as tile
    from concourse import bacc

    dt = mybir.dt
    mdt = dt.bfloat16
    nc = bacc.Bacc("TRN2", target_bir_lowering=False, debug=False,
                   num_devices=1 if sim else N_CORES)

    # pre-tiled host layouts: partition dim first, (ko, n) contiguous
    xtr_in = nc.declare_dram_parameter("xtr", [P, KT, R], mdt, isOutput=False)
    xtc_in = nc.declare_dram_parameter("xtc", [P, KT, R], mdt, isOutput=False)
    p_in = nc.declare_dram_parameter("p", [P, KT, I], mdt, isOutput=False)
    qt_in = nc.declare_dram_parameter("qt", [P, KT, I], mdt, isOutput=False)
    mask_in = nc.declare_dram_parameter("mask", [P, I], dt.float32,
                                        isOutput=False)
    out = nc.declare_dram_parameter("out", [R, D], mdt, isOutput=True)

    out_ap = out.ap().rearrange("(mo mi) n -> mi mo n", mi=P)  # [128, 4, 4096]

    with tile.TileContext(nc) as tc:
        with (
            tc.tile_pool(name="wres", bufs=1) as wres_pool,
            tc.tile_pool(name="xc", bufs=1) as xc_pool,
            tc.tile_pool(name="xr", bufs=1) as xr_pool,
            tc.tile_pool(name="ab", bufs=1) as ab_pool,
            tc.tile_pool(name="bj", bufs=bj_bufs) as bj_pool,
            tc.tile_pool(name="oc", bufs=oc_bufs) as oc_pool,
            tc.tile_pool(name="msk", bufs=1) as msk_pool,
            tc.tile_pool(name="ps", bufs=1, space="PSUM") as ps_pool,
            tc.tile_pool(name="dram", bufs=1, space="DRAM") as dram_pool,
        ):
            qt_sb = p_sb = mask_sb = None
            for _rep in range(repeat):
                if _rep == 0:
                    # resident across reps: P, Q weights + mask (one DMA)
                    qt_sb = wres_pool.tile([P, KT, I], mdt, name="qt",
                                           tag="qt")
                    p_sb = wres_pool.tile([P, KT, I], mdt, name="p", tag="p")
                    mask_sb = msk_pool.tile([P, I], dt.float32, name="mask",
                                            tag="mask")
                    nc.sync.dma_start(out=qt_sb[:], in_=qt_in.ap())
                    nc.sync.dma_start(out=p_sb[:], in_=p_in.ap())
                    nc.sync.dma_start(out=mask_sb[:], in_=mask_in.ap())

                if _rep == 0 and warmup_mms:
                    wu = xr_pool.tile([P, R], mdt, name="wu", tag="wu")
                    nc.any.memzero(wu[:])
                    wps = ps_pool.tile([P, R], dt.float32, name="wps",
                                       tag="ps7")
                    for _i in range(warmup_mms):
                        nc.tensor.matmul(wps[:], lhsT=wu[:, :P], rhs=wu[:],
                                         start=(_i == 0),
                                         stop=(_i == warmup_mms - 1))

                at_sb = ab_pool.tile([P, IT, R], mdt, name="at", tag="at")
                b_sb = ab_pool.tile([P, IT, R], mdt, name="b", tag="b")

                # X col tiles: resident across all B quarters, pair-loaded
                xc_sbs = []
                for k2 in range(KT // 2):
                    xc = xc_pool.tile([P, 2, R], mdt, name=f"xc{k2}",
                                      tag=f"xc{k2}")
                    nc.sync.dma_start(out=xc[:], in_=xtc_in.ap()[:, 2 * k2:2 * k2 + 2, :])
                    xc_sbs.append(xc)

                # ---- Phase B (four quarter passes) + chunked AllGather ----
                balls = []
                for q in range(4):
                    psums = [
                        ps_pool.tile([P, R], dt.float32, name=f"psb{q}{m}",
                                     tag=f"ps{2 * q + m}")
                        for m in range(2)
                    ]
                    for k in range(KT):
                        for m in range(2):
                            mg = 2 * q + m
                            nc.tensor.matmul(
                                psums[m][:],
                                lhsT=qt_sb[:, k, mg * P:(mg + 1) * P],
                                rhs=xc_sbs[k // 2][:, k % 2, :],
                                start=(k == 0),
                                stop=(k == KT - 1),
                            )
                    for m in range(2):
                        nc.vector.tensor_copy(out=b_sb[:, 2 * q + m, :],
                                              in_=psums[m][:])
                    bloc = dram_pool.tile([P, 2, R], mdt, name=f"bloc{q}",
                                          tag=f"bloc{q}")
                    ball = dram_pool.tile([N_CORES, P, 2, R], mdt,
                                          name=f"ball{q}", tag=f"ball{q}",
                                          addr_space="Local"
                                          if (sim or no_coll) else "Shared")
                    nc.sync.dma_start(out=bloc[:],
                                      in_=b_sb[:, 2 * q:2 * q + 2, :])
                    if sim or no_coll:
                        for jj in range(N_CORES):
                            nc.sync.dma_start(out=ball[jj][:1, :1, :],
                                              in_=bloc[:1, :1, :])
                    else:
                        nc.gpsimd.collective_compute(
                            "AllGather",
                            mybir.AluOpType.bypass,
                            replica_groups=[list(range(N_CORES))],
                            ins=[bloc.opt()],
                            outs=[ball.opt()],
                        )
                    balls.append(ball)

                # ---- Phase A ----
                psums = [
                    ps_pool.tile([P, R], dt.float32, name=f"psa{m}",
                                 tag=f"ps{m}")
                    for m in range(IT)
                ]
                for k2 in range(KT // 2):
                    xr = xr_pool.tile([P, 2, R], mdt,
                                      name=f"xr{k2 % xr_depth}",
                                      tag=f"xr{k2 % xr_depth}")
                    nc.sync.dma_start(out=xr[:],
                                      in_=xtr_in.ap()[:, 2 * k2:2 * k2 + 2, :])
                    for kk in range(2):
                        k = 2 * k2 + kk
                        for m in range(IT):
                            nc.tensor.matmul(
                                psums[m][:],
                                lhsT=p_sb[:, k, m * P:(m + 1) * P],
                                rhs=xr[:, kk, :],
                                start=(k == 0),
                                stop=(k == KT - 1),
                            )
                for m in range(IT):
                    nc.vector.tensor_copy(out=at_sb[:, m, :], in_=psums[m][:])

                # ---- Phase C: staircase S rows = AT^T @ B ----
                # k-major across the j's chunks: banks rotate every MM
                # instead of 8-deep same-bank runs (8-deep bank-sequential
                # accumulation measured +33%/MM vs rotation; mm_micro3).
                for j in range(N_CORES):
                    bj = bj_pool.tile([P, IT, R], mdt, name="bj", tag="bj")
                    for q in range(4):
                        nc.sync.dma_start(out=bj[:, 2 * q:2 * q + 2, :],
                                          in_=balls[q][j])
                    tlist = list(range(j // 2, MT))
                    pss = {
                        t: ps_pool.tile([P, R], dt.float32, name=f"psc{t}",
                                        tag=f"ps{t + 4 * (j % 2)}")
                        for t in tlist
                    }
                    for k in range(IT):
                        for t in tlist:
                            nc.tensor.matmul(
                                pss[t][:],
                                lhsT=at_sb[:, k, t * P:(t + 1) * P],
                                rhs=bj[:, k, :],
                                start=(k == 0),
                                stop=(k == IT - 1),
                            )
                    for t in tlist:
                        ps = pss[t]
                        ot = oc_pool.tile([P, R], mdt, name="oc", tag="oc")
                        if t == j // 2:  # diagonal block: strict-lower mask
                            half = (j % 2) * R
                            nc.vector.tensor_tensor(
                                ot[:], ps[:], mask_sb[:, half:half + R],
                                mybir.AluOpType.mult,
                            )
                        else:
                            nc.vector.tensor_copy(out=ot[:], in_=ps[:])
                        nc.sync.dma_start(out=out_ap[:, t, j * R:(j + 1) * R],
                                          in_=ot[:])

    nc.compile()
    return nc


def _make_in_maps(features: np.ndarray, Pm: np.ndarray, Qm: np.ndarray):
    features = np.asarray(features)
    X = features[0, 1:1 + D, :]
    xt = X.T.astype(BF16)                       # [feat, seq]
    p_bf = np.asarray(Pm).astype(BF16)          # [feat, inner]
    qt_bf = np.asarray(Qm).T.astype(BF16)       # [feat, inner]
    # pre-tile: (ko ki) n -> ki ko n with (ko, n) contiguous
    def pretile(a, n):
        return np.ascontiguousarray(
            a.reshape(KT, P, n).transpose(1, 0, 2))
    p_t = pretile(p_bf, I)
    qt_t = pretile(qt_bf, I)
    r_idx = np.arange(P)
    q_idx = np.arange(I)
    in_maps = []
    for c in range(N_CORES):
        mask_c = (q_idx[None, :] < (8 * r_idx[:, None] + c)).astype(np.float32)
        in_maps.append({
            "xtr": pretile(np.ascontiguousarray(xt[:, c::8]), R),
            "xtc": pretile(np.ascontiguousarray(xt[:, c * R:(c + 1) * R]), R),
            "p": p_t, "qt": qt_t, "mask": mask_c,
        })
    return in_maps


def kernel(features: np.ndarray, P: np.ndarray, Q: np.ndarray) -> np.ndarray:
    from concourse.bass_utils import run_bass_kernel_spmd

    if "nc" not in _CACHE:
        _CACHE["nc"] = _build()
    nc = _CACHE["nc"]

    in_maps = _make_in_maps(features, P, Q)
    res = run_bass_kernel_spmd(nc, in_maps, list(range(N_CORES)))
    out_full = np.empty((D, D), dtype=np.float32)
    for c in range(N_CORES):
        out_full[c::8] = res.results[c]["out"].astype(np.float32)
    return out_full


# revision 17
# speedup vs baseline: 1.0013x; 1.0013x over previous
"""ContactMapLinear Trainium2 kernel, v6.

res = tril((X @ P) @ (Q @ X^T), k=-1), X = features[0, 1:4097, :], 8-core SPMD.

v6 insight: a pure-MM microbenchmark measured ~112 ns per N=512 bf16
matmul (PE streams ~2 bf16 columns/cycle), so the kernel's PE floor is
~75 us/rep, not ~143 us -- v4's 137.6 us was DMA/collective-bound, not
compute-bound. v6 cuts steady-state HBM traffic and collective exposure:
  - P and Q weight tiles (16 MB/core) are constant across reps: loaded
    once on rep 0 into resident SBUF tiles (~128 KB/partition), no
    per-rep weight streaming.
  - Output written bf16 (host upcasts): 5.25 -> 2.6 MB/rep.
  - Phase B AllGathers B in 4 quarters (2 MB each) right after each pair
    of inner tiles, so every gather hides under the remaining ~50-29 us
    of compute (a 4 MB half-gather no longer fits under the shrunken
    phase A).
  - mask loaded rep 0 only.

  Phase B q=0..3: B[:, 2q:2q+2, :] = Qq @ Xcols_c^T,  AllGather quarter
  Phase A:        AT_c = P^T @ Xrows_c^T
  Phase C:        staircase S rows = AT^T @ B, strict-lower mask on diag.
"""

import sys

import ml_dtypes
import numpy as np

_TRN_REPO = "/opt/trn_rl_repo"
if _TRN_REPO not in sys.path:
    sys.path.insert(0, _TRN_REPO)

D = 4096          # seq length / feature dim
I = 1024          # inner dim
N_CORES = 8
R = D // N_CORES  # 512 seq rows per core
P = 128           # partitions
KT = D // P       # 32 feature k-tiles
IT = I // P       # 8 inner tiles
MT = R // P       # 4 row m-tiles per core
BF16 = ml_dtypes.bfloat16

_CACHE = {}


def _build(repeat: int = 1, sim: bool = False, bj_bufs: int = 2,
           oc_bufs: int = 3, xr_depth: int = 3, warmup_mms: int = 12,
           no_coll: bool = False, c_kmajor: bool = True,
           b_bankmajor: bool = False):
    import concourse.mybir as mybir
    import concourse.tile as tile
    from concourse import bacc

    dt = mybir.dt
    mdt = dt.bfloat16
    nc = bacc.Bacc("TRN2", target_bir_lowering=False, debug=False,
                   num_devices=1 if sim else N_CORES)

    # pre-tiled host layouts: partition dim first, (ko, n) contiguous
    xtr_in = nc.declare_dram_parameter("xtr", [P, KT, R], mdt, isOutput=False)
    xtc_in = nc.declare_dram_parameter("xtc", [P, KT, R], mdt, isOutput=False)
    p_in = nc.declare_dram_parameter("p", [P, KT, I], mdt, isOutput=False)
    qt_in = nc.declare_dram_parameter("qt", [P, KT, I], mdt, isOutput=False)
    mask_in = nc.declare_dram_parameter("mask", [P, I], dt.float32,
                                        isOutput=False)
    out = nc.declare_dram_parameter("out", [R, D], mdt, isOutput=True)

    out_ap = out.ap().rearrange("(mo mi) n -> mi mo n", mi=P)  # [128, 4, 4096]

    with tile.TileContext(nc) as tc:
        with (
            tc.tile_pool(name="wres", bufs=1) as wres_pool,
            tc.tile_pool(name="xc", bufs=1) as xc_pool,
            tc.tile_pool(name="xr", bufs=1) as xr_pool,
            tc.tile_pool(name="ab", bufs=1) as ab_pool,
            tc.tile_pool(name="bj", bufs=bj_bufs) as bj_pool,
            tc.tile_pool(name="oc", bufs=oc_bufs) as oc_pool,
            tc.tile_pool(name="msk", bufs=1) as msk_pool,
            tc.tile_pool(name="ps", bufs=1, space="PSUM") as ps_pool,
            tc.tile_pool(name="dram", bufs=1, space="DRAM") as dram_pool,
        ):
            qt_sb = p_sb = mask_sb = None
            for _rep in range(repeat):
                if _rep == 0:
                    # resident across reps: P, Q weights + mask (one DMA)
                    qt_sb = wres_pool.tile([P, KT, I], mdt, name="qt",
                                           tag="qt")
                    p_sb = wres_pool.tile([P, KT, I], mdt, name="p", tag="p")
                    mask_sb = msk_pool.tile([P, I], dt.float32, name="mask",
                                            tag="mask")
                    nc.sync.dma_start(out=qt_sb[:], in_=qt_in.ap())
                    nc.sync.dma_start(out=p_sb[:], in_=p_in.ap())
                    nc.sync.dma_start(out=mask_sb[:], in_=mask_in.ap())

                if _rep == 0 and warmup_mms:
                    wu = xr_pool.tile([P, R], mdt, name="wu", tag="wu")
                    nc.any.memzero(wu[:])
                    wps = ps_pool.tile([P, R], dt.float32, name="wps",
                                       tag="ps7")
                    for _i in range(warmup_mms):
                        nc.tensor.matmul(wps[:], lhsT=wu[:, :P], rhs=wu[:],
                                         start=(_i == 0),
                                         stop=(_i == warmup_mms - 1))

                at_sb = ab_pool.tile([P, IT, R], mdt, name="at", tag="at")
                b_sb = ab_pool.tile([P, IT, R], mdt, name="b", tag="b")

                # X col tiles: resident across all B quarters, pair-loaded
                xc_sbs = []
                for k2 in range(KT // 2):
                    xc = xc_pool.tile([P, 2, R], mdt, name=f"xc{k2}",
                                      tag=f"xc{k2}")
                    nc.sync.dma_start(out=xc[:], in_=xtc_in.ap()[:, 2 * k2:2 * k2 + 2, :])
                    xc_sbs.append(xc)

                # ---- Phase B (four quarter passes) + chunked AllGather ----
                balls = []
                for q in range(4):
                    psums = [
                        ps_pool.tile([P, R], dt.float32, name=f"psb{q}{m}",
                                     tag=f"ps{2 * q + m}")
                        for m in range(2)
                    ]
                    border = ([(m, k) for m in range(2) for k in range(KT)]
                              if b_bankmajor else
                              [(m, k) for k in range(KT) for m in range(2)])
                    for (m, k) in border:
                        mg = 2 * q + m
                        nc.tensor.matmul(
                            psums[m][:],
                            lhsT=qt_sb[:, k, mg * P:(mg + 1) * P],
                            rhs=xc_sbs[k // 2][:, k % 2, :],
                            start=(k == 0),
                            stop=(k == KT - 1),
                        )
                    for m in range(2):
                        nc.vector.tensor_copy(out=b_sb[:, 2 * q + m, :],
                                              in_=psums[m][:])
                    bloc = dram_pool.tile([P, 2, R], mdt, name=f"bloc{q}",
                                          tag=f"bloc{q}")
                    ball = dram_pool.tile([N_CORES, P, 2, R], mdt,
                                          name=f"ball{q}", tag=f"ball{q}",
                                          addr_space="Local"
                                          if (sim or no_coll) else "Shared")
                    nc.sync.dma_start(out=bloc[:],
                                      in_=b_sb[:, 2 * q:2 * q + 2, :])
                    if sim or no_coll:
                        for jj in range(N_CORES):
                            nc.sync.dma_start(out=ball[jj][:1, :1, :],
                                              in_=bloc[:1, :1, :])
                    else:
                        nc.gpsimd.collective_compute(
                            "AllGather",
                            mybir.AluOpType.bypass,
                            replica_groups=[list(range(N_CORES))],
                            ins=[bloc.opt()],
                            outs=[ball.opt()],
                        )
                    balls.append(ball)

                # ---- Phase A ----
                psums = [
                    ps_pool.tile([P, R], dt.float32, name=f"psa{m}",
                                 tag=f"ps{m}")
                    for m in range(IT)
                ]
                for k2 in range(KT // 2):
                    xr = xr_pool.tile([P, 2, R], mdt,
                                      name=f"xr{k2 % xr_depth}",
                                      tag=f"xr{k2 % xr_depth}")
                    nc.sync.dma_start(out=xr[:],
                                      in_=xtr_in.ap()[:, 2 * k2:2 * k2 + 2, :])
                    for kk in range(2):
                        k = 2 * k2 + kk
                        for m in range(IT):
                            nc.tensor.matmul(
                                psums[m][:],
                                lhsT=p_sb[:, k, m * P:(m + 1) * P],
                                rhs=xr[:, kk, :],
                                start=(k == 0),
                                stop=(k == KT - 1),
                            )
                for m in range(IT):
                    nc.vector.tensor_copy(out=at_sb[:, m, :], in_=psums[m][:])

                # ---- Phase C: staircase S rows = AT^T @ B ----
                # k-major across the j's chunks: banks rotate every MM
                # instead of 8-deep same-bank runs (8-deep bank-sequential
                # accumulation measured +33%/MM vs rotation; mm_micro3).
                for j in range(N_CORES):
                    bj = bj_pool.tile([P, IT, R], mdt, name="bj", tag="bj")
                    for q in range(4):
                        nc.sync.dma_start(out=bj[:, 2 * q:2 * q + 2, :],
                                          in_=balls[q][j])
                    tlist = list(range(j // 2, MT))
                    pss = {
                        t: ps_pool.tile([P, R], dt.float32, name=f"psc{t}",
                                        tag=f"ps{t + 4 * (j % 2)}")
                        for t in tlist
                    }
                    mm_order = ([(k, t) for k in range(IT) for t in tlist]
                                if c_kmajor else
                                [(k, t) for t in tlist for k in range(IT)])
                    for (k, t) in mm_order:
                        nc.tensor.matmul(
                            pss[t][:],
                            lhsT=at_sb[:, k, t * P:(t + 1) * P],
                            rhs=bj[:, k, :],
                            start=(k == 0),
                            stop=(k == IT - 1),
                        )
                    for t in tlist:
                        ps = pss[t]
                        ot = oc_pool.tile([P, R], mdt, name="oc", tag="oc")
                        if t == j // 2:  # diagonal block: strict-lower mask
                            half = (j % 2) * R
                            nc.vector.tensor_tensor(
                                ot[:], ps[:], mask_sb[:, half:half + R],
                                mybir.AluOpType.mult,
                            )
                        else:
                            nc.vector.tensor_copy(out=ot[:], in_=ps[:])
                        nc.sync.dma_start(out=out_ap[:, t, j * R:(j + 1) * R],
                                          in_=ot[:])

    nc.compile()
    return nc


def _make_in_maps(features: np.ndarray, Pm: np.ndarray, Qm: np.ndarray):
    features = np.asarray(features)
    X = features[0, 1:1 + D, :]
    xt = X.T.astype(BF16)                       # [feat, seq]
    p_bf = np.asarray(Pm).astype(BF16)          # [feat, inner]
    qt_bf = np.asarray(Qm).T.astype(BF16)       # [feat, inner]
    # pre-tile: (ko ki) n -> ki ko n with (ko, n) contiguous
    def pretile(a, n):
        return np.ascontiguousarray(
            a.reshape(KT, P, n).transpose(1, 0, 2))
    p_t = pretile(p_bf, I)
    qt_t = pretile(qt_bf, I)
    r_idx = np.arange(P)
    q_idx = np.arange(I)
    in_maps = []
    for c in range(N_CORES):
        mask_c = (q_idx[None, :] < (8 * r_idx[:, None] + c)).astype(np.float32)
        in_maps.append({
            "xtr": pretile(np.ascontiguousarray(xt[:, c::8]), R),
            "xtc": pretile(np.ascontiguousarray(xt[:, c * R:(c + 1) * R]), R),
            "p": p_t, "qt": qt_t, "mask": mask_c,
        })
    return in_maps


def kernel(features: np.ndarray, P: np.ndarray, Q: np.ndarray) -> np.ndarray:
    from concourse.bass_utils import run_bass_kernel_spmd

    if "nc" not in _CACHE:
        _CACHE["nc"] = _build()
    nc = _CACHE["nc"]

    in_maps = _make_in_maps(features, P, Q)
    res = run_bass_kernel_spmd(nc, in_maps, list(range(N_CORES)))
    out_full = np.empty((D, D), dtype=np.float32)
    for c in range(N_CORES):
        out_full[c::8] = res.results[c]["out"].astype(np.float32)
    return out_full


# revision 18
# speedup vs baseline: 1.9418x; 1.9393x over previous
"""ContactMapLinear Trainium2 kernel, v6.

res = tril((X @ P) @ (Q @ X^T), k=-1), X = features[0, 1:4097, :], 8-core SPMD.

v6 insight: a pure-MM microbenchmark measured ~112 ns per N=512 bf16
matmul (PE streams ~2 bf16 columns/cycle), so the kernel's PE floor is
~75 us/rep, not ~143 us -- v4's 137.6 us was DMA/collective-bound, not
compute-bound. v6 cuts steady-state HBM traffic and collective exposure:
  - P and Q weight tiles (16 MB/core) are constant across reps: loaded
    once on rep 0 into resident SBUF tiles (~128 KB/partition), no
    per-rep weight streaming.
  - Output written bf16 (host upcasts): 5.25 -> 2.6 MB/rep.
  - Phase B AllGathers B in 4 quarters (2 MB each) right after each pair
    of inner tiles, so every gather hides under the remaining ~50-29 us
    of compute (a 4 MB half-gather no longer fits under the shrunken
    phase A).
  - mask loaded rep 0 only.

  Phase B q=0..3: B[:, 2q:2q+2, :] = Qq @ Xcols_c^T,  AllGather quarter
  Phase A:        AT_c = P^T @ Xrows_c^T
  Phase C:        staircase S rows = AT^T @ B, strict-lower mask on diag.
"""

import sys

import ml_dtypes
import numpy as np

_TRN_REPO = "/opt/trn_rl_repo"
if _TRN_REPO not in sys.path:
    sys.path.insert(0, _TRN_REPO)

D = 4096          # seq length / feature dim
I = 1024          # inner dim
N_CORES = 8
R = D // N_CORES  # 512 seq rows per core
P = 128           # partitions
KT = D // P       # 32 feature k-tiles
IT = I // P       # 8 inner tiles
MT = R // P       # 4 row m-tiles per core
BF16 = ml_dtypes.bfloat16

_CACHE = {}


def _build(repeat: int = 1, sim: bool = False, bj_bufs: int = 2,
           oc_bufs: int = 3, xr_depth: int = 3, warmup_mms: int = 12,
           no_coll: bool = False, c_kmajor: bool = True,
           b_bankmajor: bool = False, stream_p: bool = False):
    import concourse.mybir as mybir
    import concourse.tile as tile
    from concourse import bacc

    dt = mybir.dt
    mdt = dt.bfloat16
    nc = bacc.Bacc("TRN2", target_bir_lowering=False, debug=False,
                   num_devices=1 if sim else N_CORES)

    # pre-tiled host layouts: partition dim first, (ko, n) contiguous
    xtr_in = nc.declare_dram_parameter("xtr", [P, KT, R], mdt, isOutput=False)
    xtc_in = nc.declare_dram_parameter("xtc", [P, KT, R], mdt, isOutput=False)
    p_in = nc.declare_dram_parameter("p", [P, KT, I], mdt, isOutput=False)
    qt_in = nc.declare_dram_parameter("qt", [P, KT, I], mdt, isOutput=False)
    mask_in = nc.declare_dram_parameter("mask", [P, I], dt.float32,
                                        isOutput=False)
    out = nc.declare_dram_parameter("out", [R, D], mdt, isOutput=True)

    out_ap = out.ap().rearrange("(mo mi) n -> mi mo n", mi=P)  # [128, 4, 4096]

    with tile.TileContext(nc) as tc:
        with (
            tc.tile_pool(name="wres", bufs=1) as wres_pool,
            tc.tile_pool(name="xc", bufs=1) as xc_pool,
            tc.tile_pool(name="xr", bufs=1) as xr_pool,
            tc.tile_pool(name="w", bufs=4) as w_pool,
            tc.tile_pool(name="ab", bufs=1) as ab_pool,
            tc.tile_pool(name="bj", bufs=bj_bufs) as bj_pool,
            tc.tile_pool(name="oc", bufs=oc_bufs) as oc_pool,
            tc.tile_pool(name="msk", bufs=1) as msk_pool,
            tc.tile_pool(name="ps", bufs=1, space="PSUM") as ps_pool,
            tc.tile_pool(name="dram", bufs=1, space="DRAM") as dram_pool,
        ):
            qt_sb = p_sb = mask_sb = None
            for _rep in range(repeat):
                if _rep == 0:
                    # resident across reps: P, Q weights + mask (one DMA)
                    qt_sb = wres_pool.tile([P, KT, I], mdt, name="qt",
                                           tag="qt")
                    mask_sb = msk_pool.tile([P, I], dt.float32, name="mask",
                                            tag="mask")
                    nc.sync.dma_start(out=qt_sb[:], in_=qt_in.ap())
                    nc.sync.dma_start(out=mask_sb[:], in_=mask_in.ap())
                    if not stream_p:
                        p_sb = wres_pool.tile([P, KT, I], mdt, name="p",
                                              tag="p")
                        nc.sync.dma_start(out=p_sb[:], in_=p_in.ap())

                if _rep == 0 and warmup_mms:
                    wu = xr_pool.tile([P, R], mdt, name="wu", tag="wu")
                    nc.any.memzero(wu[:])
                    wps = ps_pool.tile([P, R], dt.float32, name="wps",
                                       tag="ps7")
                    for _i in range(warmup_mms):
                        nc.tensor.matmul(wps[:], lhsT=wu[:, :P], rhs=wu[:],
                                         start=(_i == 0),
                                         stop=(_i == warmup_mms - 1))

                at_sb = ab_pool.tile([P, IT, R], mdt, name="at", tag="at")
                b_sb = ab_pool.tile([P, IT, R], mdt, name="b", tag="b")

                # X col tiles: resident across all B quarters, pair-loaded
                xc_sbs = []
                for k2 in range(KT // 2):
                    xc = xc_pool.tile([P, 2, R], mdt, name=f"xc{k2}",
                                      tag=f"xc{k2}")
                    nc.sync.dma_start(out=xc[:], in_=xtc_in.ap()[:, 2 * k2:2 * k2 + 2, :])
                    xc_sbs.append(xc)

                # ---- Phase B (four quarter passes) + chunked AllGather ----
                balls = []
                for q in range(4):
                    psums = [
                        ps_pool.tile([P, R], dt.float32, name=f"psb{q}{m}",
                                     tag=f"ps{2 * q + m}")
                        for m in range(2)
                    ]
                    border = ([(m, k) for m in range(2) for k in range(KT)]
                              if b_bankmajor else
                              [(m, k) for k in range(KT) for m in range(2)])
                    for (m, k) in border:
                        mg = 2 * q + m
                        nc.tensor.matmul(
                            psums[m][:],
                            lhsT=qt_sb[:, k, mg * P:(mg + 1) * P],
                            rhs=xc_sbs[k // 2][:, k % 2, :],
                            start=(k == 0),
                            stop=(k == KT - 1),
                        )
                    for m in range(2):
                        nc.vector.tensor_copy(out=b_sb[:, 2 * q + m, :],
                                              in_=psums[m][:])
                    bloc = dram_pool.tile([P, 2, R], mdt, name=f"bloc{q}",
                                          tag=f"bloc{q}")
                    ball = dram_pool.tile([N_CORES, P, 2, R], mdt,
                                          name=f"ball{q}", tag=f"ball{q}",
                                          addr_space="Local"
                                          if (sim or no_coll) else "Shared")
                    nc.sync.dma_start(out=bloc[:],
                                      in_=b_sb[:, 2 * q:2 * q + 2, :])
                    if sim or no_coll:
                        for jj in range(N_CORES):
                            nc.sync.dma_start(out=ball[jj][:1, :1, :],
                                              in_=bloc[:1, :1, :])
                    else:
                        nc.gpsimd.collective_compute(
                            "AllGather",
                            mybir.AluOpType.bypass,
                            replica_groups=[list(range(N_CORES))],
                            ins=[bloc.opt()],
                            outs=[ball.opt()],
                        )
                    balls.append(ball)

                # ---- Phase A ----
                psums = [
                    ps_pool.tile([P, R], dt.float32, name=f"psa{m}",
                                 tag=f"ps{m}")
                    for m in range(IT)
                ]
                for k2 in range(KT // 2):
                    xr = xr_pool.tile([P, 2, R], mdt,
                                      name=f"xr{k2 % xr_depth}",
                                      tag=f"xr{k2 % xr_depth}")
                    nc.sync.dma_start(out=xr[:],
                                      in_=xtr_in.ap()[:, 2 * k2:2 * k2 + 2, :])
                    if stream_p:
                        w_sb = w_pool.tile([P, 2, I], mdt, name="wp",
                                           tag="w")
                        nc.sync.dma_start(
                            out=w_sb[:],
                            in_=p_in.ap()[:, 2 * k2:2 * k2 + 2, :])
                    for kk in range(2):
                        k = 2 * k2 + kk
                        for m in range(IT):
                            nc.tensor.matmul(
                                psums[m][:],
                                lhsT=(w_sb[:, kk, m * P:(m + 1) * P]
                                      if stream_p else
                                      p_sb[:, k, m * P:(m + 1) * P]),
                                rhs=xr[:, kk, :],
                                start=(k == 0),
                                stop=(k == KT - 1),
                            )
                for m in range(IT):
                    nc.vector.tensor_copy(out=at_sb[:, m, :], in_=psums[m][:])

                # ---- Phase C: staircase S rows = AT^T @ B ----
                # k-major across the j's chunks: banks rotate every MM
                # instead of 8-deep same-bank runs (8-deep bank-sequential
                # accumulation measured +33%/MM vs rotation; mm_micro3).
                for j in range(N_CORES):
                    bj = bj_pool.tile([P, IT, R], mdt, name="bj", tag="bj")
                    for q in range(4):
                        nc.sync.dma_start(out=bj[:, 2 * q:2 * q + 2, :],
                                          in_=balls[q][j])
                    tlist = list(range(j // 2, MT))
                    pss = {
                        t: ps_pool.tile([P, R], dt.float32, name=f"psc{t}",
                                        tag=f"ps{t + 4 * (j % 2)}")
                        for t in tlist
                    }
                    mm_order = ([(k, t) for k in range(IT) for t in tlist]
                                if c_kmajor else
                                [(k, t) for t in tlist for k in range(IT)])
                    for (k, t) in mm_order:
                        nc.tensor.matmul(
                            pss[t][:],
                            lhsT=at_sb[:, k, t * P:(t + 1) * P],
                            rhs=bj[:, k, :],
                            start=(k == 0),
                            stop=(k == IT - 1),
                        )
                    for t in tlist:
                        ps = pss[t]
                        ot = oc_pool.tile([P, R], mdt, name="oc", tag="oc")
                        if t == j // 2:  # diagonal block: strict-lower mask
                            half = (j % 2) * R
                            nc.vector.tensor_tensor(
                                ot[:], ps[:], mask_sb[:, half:half + R],
                                mybir.AluOpType.mult,
                            )
                        else:
                            nc.vector.tensor_copy(out=ot[:], in_=ps[:])
                        nc.sync.dma_start(out=out_ap[:, t, j * R:(j + 1) * R],
                                          in_=ot[:])

    nc.compile()
    return nc


def _make_in_maps(features: np.ndarray, Pm: np.ndarray, Qm: np.ndarray):
    features = np.asarray(features)
    X = features[0, 1:1 + D, :]
    xt = X.T.astype(BF16)                       # [feat, seq]
    p_bf = np.asarray(Pm).astype(BF16)          # [feat, inner]
    qt_bf = np.asarray(Qm).T.astype(BF16)       # [feat, inner]
    # pre-tile: (ko ki) n -> ki ko n with (ko, n) contiguous
    def pretile(a, n):
        return np.ascontiguousarray(
            a.reshape(KT, P, n).transpose(1, 0, 2))
    p_t = pretile(p_bf, I)
    qt_t = pretile(qt_bf, I)
    r_idx = np.arange(P)
    q_idx = np.arange(I)
    in_maps = []
    for c in range(N_CORES):
        mask_c = (q_idx[None, :] < (8 * r_idx[:, None] + c)).astype(np.float32)
        in_maps.append({
            "xtr": pretile(np.ascontiguousarray(xt[:, c::8]), R),
            "xtc": pretile(np.ascontiguousarray(xt[:, c * R:(c + 1) * R]), R),
            "p": p_t, "qt": qt_t, "mask": mask_c,
        })
    return in_maps


def kernel(features: np.ndarray, P: np.ndarray, Q: np.ndarray) -> np.ndarray:
    from concourse.bass_utils import run_bass_kernel_spmd

    if "nc" not in _CACHE:
        _CACHE["nc"] = _build()
    nc = _CACHE["nc"]

    in_maps = _make_in_maps(features, P, Q)
    res = run_bass_kernel_spmd(nc, in_maps, list(range(N_CORES)))
    out_full = np.empty((D, D), dtype=np.float32)
    for c in range(N_CORES):
        out_full[c::8] = res.results[c]["out"].astype(np.float32)
    return out_full
